# revision 1
# baseline (speedup 1.0000x reference)
"""Trainium2 Bass kernel: dense multi-head attention layer (B=4, L=S=2048,
d_model=1024, 16 heads x 64).

Sharding: 8 cores = (batch b in 0..3) x (query-half in 0..1). Each core runs
the full 16-head attention for its 1024 query rows (K/V projection duplicated
between the two cores sharing a batch) plus the out-projection for those rows,
so no collectives are needed and each core emits complete output rows.

Layout trick: inputs are pre-transposed on the host to [d_model, pos] so every
matmul in the chain is natural for the PE array (contraction on partitions):
  kT = Wk.T @ XkT          (lhsT=Wk,  rhs=XkT)        -> [fout, pos]
  qT = Wq.T @ XqT          (lhsT=Wq,  rhs=XqT)        -> [fout, pos]
  v  = XvT.T @ Wv          (lhsT=XvT, rhs=Wv)         -> [pos, fout]
  scoresT_h = kT_h.T @ qT_h  (K=64, head pairs row-packed) -> [S, L]
  exp on ACT (no max subtraction needed: logits ~ N(0,1))
  avT_h = [v_h | 1].T @ expT_h  (K=S tiles, M=65)     -> [E+1, L], row 64 = sums
  outT_h = avT_h[0:64] * (1/sums broadcast)           -> [E, L]
  final = outT.T @ Wo (lhsT=outT, rhs=Wo)             -> [L, fout]  natural!
Biases enter via per-partition DVE adds (bq, bk) or K=1 ones-row matmuls
(bv, bo). Softmax scale 1/8 is folded into the ACT exp scale.

Phase order K -> Q -> V with double-buffered streaming input pools keeps the
PE dense (no pool-boundary DMA stalls -> no HAM re-throttle), and the scores
for head-pair 0 are interleaved into the V stage so the ACT exp pipeline
starts ~100us earlier.
"""

import os

import ml_dtypes
import numpy as np

B, L, S, DM, H, E = 4, 2048, 2048, 1024, 16, 64
P = 128
FT = DM // P          # 8 feature tiles of 128
LL = L // 2           # 1024 query rows per core
NST = S // P          # 16 key/value position tiles
NH2 = H // 2          # 8 head pairs
W = E + 1             # v columns + ones column for the softmax denominator
NCORES = 8

_graph_cache = {}


def _build_graph():
    if "nc" in _graph_cache:
        return _graph_cache["nc"]

    import concourse.tile as tile
    from concourse import bacc, mybir

    BF16 = mybir.dt.bfloat16
    F32 = mybir.dt.float32
    Exp = mybir.ActivationFunctionType.Exp

    nc = bacc.Bacc("TRN2", target_bir_lowering=False, debug=False,
                   num_devices=NCORES)

    xqT_d = nc.declare_dram_parameter("xqT", [DM, LL], BF16, isOutput=False)
    xkT_d = nc.declare_dram_parameter("xkT", [DM, S], BF16, isOutput=False)
    xvT_d = nc.declare_dram_parameter("xvT", [DM, S], BF16, isOutput=False)
    wq_d = nc.declare_dram_parameter("wq", [DM, DM], BF16, isOutput=False)
    wk_d = nc.declare_dram_parameter("wk", [DM, DM], BF16, isOutput=False)
    wv_d = nc.declare_dram_parameter("wv", [DM, DM], BF16, isOutput=False)
    wo_d = nc.declare_dram_parameter("wo", [DM, DM], BF16, isOutput=False)
    bq_d = nc.declare_dram_parameter("bq", [DM], F32, isOutput=False)
    bk_d = nc.declare_dram_parameter("bk", [DM], F32, isOutput=False)
    bv_d = nc.declare_dram_parameter("bv", [DM], BF16, isOutput=False)
    bo_d = nc.declare_dram_parameter("bo", [DM], BF16, isOutput=False)
    out_d = nc.declare_dram_parameter("out", [LL, DM], F32, isOutput=True)

    NC_Q = LL // 512      # 2 query-position chunks of 512
    HS = S // 2           # 1024 positions per streamed input half

    with tile.TileContext(nc) as tc:
        with tc.tile_pool(name="persist", bufs=1) as pp, \
             tc.tile_pool(name="norm", bufs=2) as npool, \
             tc.tile_pool(name="expp", bufs=8) as expp:
            qT = pp.tile([P, FT, LL], BF16)
            kT = pp.tile([P, FT, S], BF16)
            v_sb = pp.tile([P, NST, H * W], BF16)
            bq_sb = pp.tile([P, FT], F32)
            bk_sb = pp.tile([P, FT], F32)
            bv_sb = pp.tile([1, DM], BF16)
            bo_sb = pp.tile([1, DM], BF16)
            ones_bf = pp.tile([1, P], BF16)

            nc.vector.memset(ones_bf, 1.0)
            v4 = v_sb[:].rearrange("p s (h w) -> p s h w", w=W)
            nc.vector.memset(v4[:, :, :, E:W], 1.0)

            nc.sync.dma_start(out=bq_sb[:],
                              in_=bq_d.ap().rearrange("(f p) -> p f", p=P))
            nc.sync.dma_start(out=bk_sb[:],
                              in_=bk_d.ap().rearrange("(f p) -> p f", p=P))
            nc.sync.dma_start(out=bv_sb[:],
                              in_=bv_d.ap().rearrange("(a d) -> a d", a=1))
            nc.sync.dma_start(out=bo_sb[:],
                              in_=bo_d.ap().rearrange("(a d) -> a d", a=1))

            # Scores + exp for one (head-pair, chunk, half-of-S) block set.
            with tc.tile_pool(name="ps_sc", bufs=2, space="PSUM") as ps_sc:

                # PE warmup: dummy K=1 matmuls with no DMA dependency keep
                # the PE busy through the input-DMA ramp so the HAM clock
                # gate is already at 8/8 when the first real matmul issues.
                warm = ps_sc.tile([P, 2, 512], F32, tag="sc", name="warm")
                for wi in range(28):
                    nc.tensor.matmul(warm[:, 0, 0:P],
                                     lhsT=ones_bf[:, 0:P],
                                     rhs=ones_bf[:, 0:P],
                                     start=True, stop=True)

                def score_block(j, c, half, sb):
                    # Both heads' scores for one S-tile share ONE psum tile
                    # (single slot -> simultaneous readiness -> the scheduler
                    # keeps the two row-group-disjoint MMs adjacent -> the PE
                    # array runs them concurrently). Each block owns a small
                    # ex tile (2 S-tiles) so AV dependencies are precise and
                    # slots rotate tile-by-tile.
                    ex = expp.tile([P, 2, 2, 512], BF16, tag="ex",
                                   name=f"ex_{j}_{c}_{half}_{sb}")
                    for si in range(2):
                        s = half * 8 + sb * 2 + si
                        scp = ps_sc.tile([P, 2, 512], F32, tag="sc",
                                         name=f"sc_{j}_{c}_{half}_{sb}_{si}")
                        for r in range(2):
                            nc.tensor.matmul(
                                scp[:, r, :],
                                lhsT=kT[r * E:(r + 1) * E, j,
                                        s * P:(s + 1) * P],
                                rhs=qT[r * E:(r + 1) * E, j,
                                       c * 512:(c + 1) * 512],
                                start=True, stop=True)
                        nc.scalar.activation(
                            out=ex[:, :, si, :],
                            in_=scp[:], func=Exp, scale=0.125)
                    return ex

                # ------------- Phase A: K -> Q -> scores(0,0) -> V --------
                with tc.tile_pool(name="stA", bufs=1) as stA, \
                     tc.tile_pool(name="psA", bufs=4, space="PSUM") as psA:

                    def w_tile(nm, dram):
                        t = stA.tile([P, FT, DM], BF16, tag="w", bufs=2,
                                     name=nm)
                        src_ap = dram.ap().rearrange("(f p) n -> p f n", p=P)
                        for f in range(FT):
                            nc.sync.dma_start(out=t[:, f], in_=src_ap[:, f])
                        return t

                    def x_tile(nm, dram, lo, n):
                        t = stA.tile([P, FT, n], BF16, tag="x", bufs=2,
                                     name=nm)
                        src_ap = (dram.ap()[:, lo:lo + n]
                                  .rearrange("(f p) n -> p f n", p=P))
                        for f in range(FT):
                            nc.sync.dma_start(out=t[:, f], in_=src_ap[:, f])
                        return t

                    wk_sb = w_tile("wk", wk_d)
                    xk0 = x_tile("xk0", xkT_d, 0, HS)
                    xk1 = x_tile("xk1", xkT_d, HS, HS)

                    def proj_qk(x_sb, w_sb, dst, bias, pos0, npos, tagn):
                        for m in range(FT):
                            pss = [psA.tile([P, 512], F32, tag="psa",
                                            name=f"psa_{tagn}_{m}_{c}")
                                   for c in range(npos // 512)]
                            for f in range(FT):
                                for c in range(npos // 512):
                                    nc.tensor.matmul(
                                        pss[c][:],
                                        lhsT=w_sb[:, f, m * P:(m + 1) * P],
                                        rhs=x_sb[:, f,
                                                 c * 512:(c + 1) * 512],
                                        start=(f == 0), stop=(f == FT - 1))
                            for c in range(npos // 512):
                                nc.vector.tensor_scalar_add(
                                    dst[:, m,
                                        pos0 + c * 512:pos0 + (c + 1) * 512],
                                    pss[c][:], bias[:, m:m + 1])

                    proj_qk(xk0, wk_sb, kT, bk_sb, 0, HS, "k0")
                    wq_sb = w_tile("wq", wq_d)
                    proj_qk(xk1, wk_sb, kT, bk_sb, HS, HS, "k1")
                    xq_sb = x_tile("xq", xqT_d, 0, LL)
                    proj_qk(xq_sb, wq_sb, qT, bq_sb, 0, LL, "q")
                    wv_sb = w_tile("wv", wv_d)

                    # scores for head-pair 0, chunk 0: ACT ramps while the
                    # V projection keeps the PE dense.
                    ex_00 = [score_block(0, 0, hf, sb)
                             for hf in range(2) for sb in range(4)]

                    for vh in range(2):
                        xv_h = x_tile(f"xv{vh}", xvT_d, vh * HS, HS)
                        for ti in range(8):
                            t = vh * 8 + ti
                            pss = [psA.tile([P, 512], F32, tag="psa",
                                            name=f"psa_v{t}_{c}")
                                   for c in range(2)]
                            for f in range(FT):
                                for c in range(2):
                                    nc.tensor.matmul(
                                        pss[c][:],
                                        lhsT=xv_h[:, f, ti * P:(ti + 1) * P],
                                        rhs=wv_sb[:, f,
                                                  c * 512:(c + 1) * 512],
                                        start=(f == 0), stop=False)
                            for c in range(2):
                                nc.tensor.matmul(
                                    pss[c][:],
                                    lhsT=ones_bf[:, 0:P],
                                    rhs=bv_sb[:, c * 512:(c + 1) * 512],
                                    start=False, stop=True)
                                nc.vector.tensor_copy(
                                    out=v4[:, t, c * 8:(c + 1) * 8, 0:E],
                                    in_=pss[c][:].rearrange(
                                        "p (h e) -> p h e", e=E))

                # ------------- Phase B (+ interleaved phase C) -------------
                with tc.tile_pool(name="bo_pool", bufs=1) as bop, \
                     tc.tile_pool(name="osb", bufs=2) as osb:
                    _av_cm = tc.tile_pool(name="ps_av", bufs=3, space="PSUM")
                    ps_av = _av_cm.__enter__()
                    _pc_cm = tc.tile_pool(name="psC", bufs=1, space="PSUM")
                    psC = _pc_cm.__enter__()
                    outT = bop.tile([P, FT, LL], BF16)
                    wo_sb = bop.tile([P, FT, DM], BF16)
                    nc.sync.dma_start(
                        out=wo_sb[:],
                        in_=wo_d.ap().rearrange("(f p) n -> p f n", p=P))

                    def out_proj(t, c2, pool):
                        ps = pool.tile([P, 512], F32, tag="psc",
                                       name=f"psc_{t}_{c2}")
                        for f in range(FT):
                            nc.tensor.matmul(
                                ps[:],
                                lhsT=outT[:, f, t * P:(t + 1) * P],
                                rhs=wo_sb[:, f, c2 * 512:(c2 + 1) * 512],
                                start=(f == 0), stop=False)
                        nc.tensor.matmul(
                            ps[:], lhsT=ones_bf[:, 0:P],
                            rhs=bo_sb[:, c2 * 512:(c2 + 1) * 512],
                            start=False, stop=True)
                        o_sb = osb.tile([P, 512], F32, tag="osb",
                                        name=f"osb_{t}_{c2}")
                        nc.vector.tensor_copy(out=o_sb[:], in_=ps[:])
                        nc.sync.dma_start(
                            out=out_d[t * P:(t + 1) * P,
                                      c2 * 512:(c2 + 1) * 512],
                            in_=o_sb[:])

                    for c in range(NC_Q):
                        for j in range(NH2):
                            avs = [ps_av.tile([W, 512], F32, tag="av",
                                              name=f"av_{j}_{c}_{r}")
                                   for r in range(2)]
                            if j == 0 and c == 0:
                                ex_tiles = ex_00
                            else:
                                ex_tiles = [score_block(j, c, hf, sb)
                                            for hf in range(2)
                                            for sb in range(4)]
                            for s in range(NST):
                                ex = ex_tiles[s // 2]
                                for r in range(2):
                                    nc.tensor.matmul(
                                        avs[r][:],
                                        lhsT=v4[:, s, 2 * j + r, :],
                                        rhs=ex[:, r, s % 2, :],
                                        start=(s == 0), stop=(s == NST - 1))
                            for r in range(2):
                                # one-shot psum evac releases the av slot
                                # ~3us earlier; the normalize chain then
                                # runs SBUF->SBUF off the slot path.
                                avsb = npool.tile([E, 512], F32, tag="avsb",
                                                  bufs=2,
                                                  name=f"avsb_{j}_{c}_{r}")
                                nc.vector.tensor_copy(out=avsb[:],
                                                      in_=avs[r][0:E, :])
                                sums = npool.tile([1, 512], F32, tag="sums",
                                                  bufs=2,
                                                  name=f"sums_{j}_{c}_{r}")
                                nc.vector.tensor_copy(out=sums[:],
                                                      in_=avs[r][E:W, :])
                                recip = npool.tile([1, 512], F32,
                                                   tag="recip",
                                                   name=f"recip_{j}_{c}_{r}")
                                nc.vector.reciprocal_approx_fast(
                                    recip[:], sums[:])
                                bc = npool.tile([E, 512], F32, tag="bc",
                                                bufs=2,
                                                name=f"bc_{j}_{c}_{r}")
                                nc.gpsimd.partition_broadcast(bc[:],
                                                              recip[:])
                                nc.vector.tensor_mul(
                                    outT[r * E:(r + 1) * E, j,
                                         c * 512:(c + 1) * 512],
                                    avsb[0:E, :], bc[:])
                            # interleave chunk-0 out-projection into the
                            # chunk-1 head loop (one L-tile per head pair)
                            if c == 1:
                                out_proj(j // 2, j % 2, psC)

                    _pc_cm.__exit__(None, None, None)
                    _av_cm.__exit__(None, None, None)
                    with tc.tile_pool(name="psC2", bufs=4,
                                      space="PSUM") as psC2:
                        for t in range(2, LL // P):
                            for c2 in range(2):
                                out_proj(t, c2, psC2)


    nc.finalize()
    _graph_cache["nc"] = nc
    return nc


def _install_profile_shim():
    """Provide antenv.axon_hooks (NTFF capture via libaxon_pjrt ctypes) when
    the image's antenv lacks it, and skip the artifact upload step."""
    import contextlib
    import ctypes
    import sys
    import types

    try:
        from antenv.axon_hooks import get_axon_ntff_profile_hook
        if get_axon_ntff_profile_hook() is not None:
            return
    except ImportError:
        pass

    so_path = "/opt/axon/libaxon_pjrt.so"
    try:
        lib = ctypes.CDLL(so_path)
    except OSError:
        return
    if not hasattr(lib, "axon_start_nrt_profile"):
        return
    lib.axon_start_nrt_profile.argtypes = [ctypes.POINTER(ctypes.c_int64),
                                           ctypes.c_size_t]
    lib.axon_start_nrt_profile.restype = ctypes.c_int64
    lib.axon_stop_nrt_profile.argtypes = [ctypes.c_char_p]
    lib.axon_stop_nrt_profile.restype = ctypes.c_int64

    @contextlib.contextmanager
    def _hook(output_dir, device_ids):
        import jax
        jax.devices()
        if device_ids:
            ids = (ctypes.c_int64 * len(device_ids))(*device_ids)
            rc = lib.axon_start_nrt_profile(ids, len(device_ids))
        else:
            rc = lib.axon_start_nrt_profile(None, 0)
        if rc != 0:
            raise RuntimeError(f"axon_start_nrt_profile rc={rc}")
        try:
            yield
        finally:
            n = lib.axon_stop_nrt_profile(str(output_dir).encode())
            print(f"profile: {n} file(s) written to {output_dir}",
                  file=sys.stderr)

    mod = types.ModuleType("antenv.axon_hooks")
    mod.get_axon_ntff_profile_hook = lambda: _hook
    mod.set_axon_ntff_profile_hook = lambda h: None
    sys.modules["antenv.axon_hooks"] = mod

    import concourse.bass_utils as bu
    bu.upload_artifacts = lambda tmpdir: str(tmpdir)


def kernel(queries, keys, values, Wq, bq, Wk, bk, Wv, bv, Wo, bo):
    from concourse.bass_utils import run_bass_kernel_spmd

    nc = _build_graph()
    bf = ml_dtypes.bfloat16

    wq_b = np.ascontiguousarray(np.asarray(Wq, np.float32).astype(bf))
    wk_b = np.ascontiguousarray(np.asarray(Wk, np.float32).astype(bf))
    wv_b = np.ascontiguousarray(np.asarray(Wv, np.float32).astype(bf))
    wo_b = np.ascontiguousarray(np.asarray(Wo, np.float32).astype(bf))
    bq_f = np.ascontiguousarray(np.asarray(bq, np.float32))
    bk_f = np.ascontiguousarray(np.asarray(bk, np.float32))
    bv_b = np.ascontiguousarray(np.asarray(bv, np.float32).astype(bf))
    bo_b = np.ascontiguousarray(np.asarray(bo, np.float32).astype(bf))

    qT = np.ascontiguousarray(
        np.transpose(np.asarray(queries, np.float32), (0, 2, 1)).astype(bf))
    kTt = np.ascontiguousarray(
        np.transpose(np.asarray(keys, np.float32), (0, 2, 1)).astype(bf))
    vTt = np.ascontiguousarray(
        np.transpose(np.asarray(values, np.float32), (0, 2, 1)).astype(bf))

    in_maps = []
    for core in range(NCORES):
        b, h = divmod(core, 2)
        in_maps.append({
            "xqT": np.ascontiguousarray(qT[b][:, h * LL:(h + 1) * LL]),
            "xkT": kTt[b],
            "xvT": vTt[b],
            "wq": wq_b, "wk": wk_b, "wv": wv_b, "wo": wo_b,
            "bq": bq_f, "bk": bk_f, "bv": bv_b, "bo": bo_b,
        })

    trace = bool(int(os.environ.get("KERNEL_PROFILE", "0")))
    if trace:
        _install_profile_shim()
    res = run_bass_kernel_spmd(nc, in_maps, core_ids=list(range(NCORES)),
                               trace=trace)
    kernel.last_results = res

    out = np.empty((B, L, DM), np.float32)
    for core in range(NCORES):
        b, h = divmod(core, 2)
        out[b, h * LL:(h + 1) * LL, :] = res.results[core]["out"]
    return out


kernel.last_results = None



# revision 26
# speedup vs baseline: 1.1658x; 1.1658x over previous
"""Trainium2 Bass kernel: dense multi-head attention layer (B=4, L=S=2048,
d_model=1024, 16 heads x 64).

Sharding: 8 cores = (batch b in 0..3) x (query-half in 0..1). Each core runs
the full 16-head attention for its 1024 query rows (K/V projection duplicated
between the two cores sharing a batch) plus the out-projection for those rows,
so no collectives are needed and each core emits complete output rows.

Layout trick: inputs are pre-transposed on the host to [d_model, pos] so every
matmul in the chain is natural for the PE array (contraction on partitions):
  kT = Wk.T @ XkT          (lhsT=Wk,  rhs=XkT)        -> [fout, pos]
  qT = Wq.T @ XqT          (lhsT=Wq,  rhs=XqT)        -> [fout, pos]
  v  = XvT.T @ Wv          (lhsT=XvT, rhs=Wv)         -> [pos, fout]
  scoresT_h = kT_h.T @ qT_h  (K=64, head pairs row-packed) -> [S, L]
  exp on ACT (no max subtraction needed: logits ~ N(0,1))
  avT_h = [v_h | 1].T @ expT_h  (K=S tiles, M=65)     -> [E+1, L], row 64 = sums
  outT_h = avT_h * (1/sums broadcast)                 -> [E, L]
  final = outT.T @ Wo (lhsT=outT, rhs=Wo)             -> [L, fout]  natural!
Biases enter via per-partition DVE adds (bq, bk) or K=1 ones-row matmuls
(bv, bo). Softmax scale 1/8 is folded into the ACT exp scale.

Schedule: the kernel's critical path is the ACT engine's exp stream (~317us
of activations) plus the PE projection ramp in front of it. Phase A is
ordered K(h0) -> Q(c0) -> scores(j0,c0,h0) -> K(h1) -> scores(j0,c0,h1) ->
Q(c1) -> V-proj, with scores j1..j2 (chunk 0) and the j0 AV accumulation
interleaved into the V projection (AV chases V-tile completion), so the ACT
exp pipeline starts at ~50us instead of ~200us and stays fed.

Notes from hardware iteration (do not re-try blindly):
- Two interleaved PSUM accumulation groups in one bank zero-region (e.g. an
  M=64 AV matmul plus an M=1 ones "sums" rider at col offset 64) are
  rejected/corrupted: start_tensor_calc clears the whole zero region.
- gpsimd.partition_broadcast of a DMA'd bias row + DVE tensor_add PSUM
  evacuation produced NaN on hardware (CoreSim-clean); the ones-row bias
  matmul form below is hardware-proven.
- Head-split sharding (8 heads/core, pairwise ReduceScatter of the
  row-parallel out-projection) shrinks the compute span to ~408us but the
  HBM collective + copy-out tail costs ~135us -> net loss vs this variant.
"""

import os

import ml_dtypes
import numpy as np

B, L, S, DM, H, E = 4, 2048, 2048, 1024, 16, 64
P = 128
FT = DM // P          # 8 feature tiles of 128
LL = L // 2           # 1024 query rows per core
NST = S // P          # 16 key/value position tiles
NH2 = H // 2          # 8 head pairs
W = E + 1             # v columns + ones column for the softmax denominator
NCORES = 8

_graph_cache = {}


def _build_graph():
    if "nc" in _graph_cache:
        return _graph_cache["nc"]

    import concourse.tile as tile
    from concourse import bacc, mybir

    BF16 = mybir.dt.bfloat16
    F32 = mybir.dt.float32
    Exp = mybir.ActivationFunctionType.Exp

    nc = bacc.Bacc("TRN2", target_bir_lowering=False, debug=False,
                   num_devices=NCORES)

    xqT_d = nc.declare_dram_parameter("xqT", [DM, LL], BF16, isOutput=False)
    xkT_d = nc.declare_dram_parameter("xkT", [DM, S], BF16, isOutput=False)
    xvT_d = nc.declare_dram_parameter("xvT", [DM, S], BF16, isOutput=False)
    wq_d = nc.declare_dram_parameter("wq", [DM, DM], BF16, isOutput=False)
    wk_d = nc.declare_dram_parameter("wk", [DM, DM], BF16, isOutput=False)
    wv_d = nc.declare_dram_parameter("wv", [DM, DM], BF16, isOutput=False)
    wo_d = nc.declare_dram_parameter("wo", [DM, DM], BF16, isOutput=False)
    bq_d = nc.declare_dram_parameter("bq", [DM], F32, isOutput=False)
    bk_d = nc.declare_dram_parameter("bk", [DM], F32, isOutput=False)
    bv_d = nc.declare_dram_parameter("bv", [DM], BF16, isOutput=False)
    bo_d = nc.declare_dram_parameter("bo", [DM], BF16, isOutput=False)
    out_d = nc.declare_dram_parameter("out", [LL, DM], F32, isOutput=True)

    HS = S // 2           # 1024 positions per streamed input half

    with tile.TileContext(nc) as tc:
        with tc.tile_pool(name="persist", bufs=1) as pp, \
             tc.tile_pool(name="norm", bufs=2) as npool, \
             tc.tile_pool(name="expp", bufs=15) as expp, \
             tc.tile_pool(name="ps_sc", bufs=2, space="PSUM") as ps_sc, \
             tc.tile_pool(name="ps_av", bufs=2, space="PSUM") as ps_av:
            qT = pp.tile([P, FT, LL], BF16)
            kT = pp.tile([P, FT, S], BF16)
            v_sb = pp.tile([P, NST, H * W], BF16)
            bq_sb = pp.tile([P, FT], F32)
            bk_sb = pp.tile([P, FT], F32)
            bv_sb = pp.tile([1, DM], BF16)
            bo_sb = pp.tile([1, DM], BF16)
            ones_bf = pp.tile([1, P], BF16)

            nc.vector.memset(ones_bf, 1.0)
            v4 = v_sb[:].rearrange("p s (h w) -> p s h w", w=W)
            nc.vector.memset(v4[:, :, :, E:W], 1.0)

            nc.sync.dma_start(out=bq_sb[:],
                              in_=bq_d.ap().rearrange("(f p) -> p f", p=P))
            nc.sync.dma_start(out=bk_sb[:],
                              in_=bk_d.ap().rearrange("(f p) -> p f", p=P))
            nc.sync.dma_start(out=bv_sb[:],
                              in_=bv_d.ap().rearrange("(a d) -> a d", a=1))
            nc.sync.dma_start(out=bo_sb[:],
                              in_=bo_d.ap().rearrange("(a d) -> a d", a=1))

            # PE warmup: dummy K=1 matmuls with no DMA dependency keep
            # the PE busy through the input-DMA ramp so the HAM clock
            # gate is already at 8/8 when the first real matmul issues.
            warm = ps_sc.tile([P, 2, 512], F32, tag="sc", name="warm")
            for wi in range(28):
                nc.tensor.matmul(warm[:, 0, 0:P],
                                 lhsT=ones_bf[:, 0:P],
                                 rhs=ones_bf[:, 0:P],
                                 start=True, stop=True)

            # Scores + exp for one (head-pair, chunk, half-of-S) block set.
            def score_block(j, c, half, sb):
                # Both heads' scores for one S-tile share ONE psum tile;
                # the two K=64 matmuls sit in disjoint row groups (base
                # partitions 0/64) so the PE runs them concurrently.
                ex = expp.tile([P, 2, 2, 512], BF16, tag="ex",
                               name=f"ex_{j}_{c}_{half}_{sb}")
                for si in range(2):
                    s = half * 8 + sb * 2 + si
                    scp = ps_sc.tile([P, 2, 512], F32, tag="sc",
                                     name=f"sc_{j}_{c}_{half}_{sb}_{si}")
                    for r in range(2):
                        nc.tensor.matmul(
                            scp[:, r, :],
                            lhsT=kT[r * E:(r + 1) * E, j,
                                    s * P:(s + 1) * P],
                            rhs=qT[r * E:(r + 1) * E, j,
                                   c * 512:(c + 1) * 512],
                            start=True, stop=True)
                    nc.scalar.activation(
                        out=ex[:, :, si, :],
                        in_=scp[:], func=Exp, scale=0.125)
                return ex

            # AV accumulation for one (head-pair, chunk) unit: M=65
            # matmuls ([v_h | 1] stationary) so row 64 of the PSUM bank
            # accumulates the softmax denominator for free.
            def av_unit(j, c, ex_tiles):
                avs = [ps_av.tile([W, 512], F32, tag="av",
                                  name=f"av_{j}_{c}_{r}")
                       for r in range(2)]
                for s in range(NST):
                    ex = ex_tiles[s // 2]
                    for r in range(2):
                        nc.tensor.matmul(
                            avs[r][:],
                            lhsT=v4[:, s, 2 * j + r, :],
                            rhs=ex[:, r, s % 2, :],
                            start=(s == 0), stop=(s == NST - 1))
                return avs

            def normalize(j, c, avs, outT):
                # NOTE: reciprocal_approx_fast (custom DVE op) gives NaN on
                # hardware when its input AP has a non-zero base partition
                # (sim-clean) — the sums row MUST be staged to a
                # base-partition-0 tile first.
                for r in range(2):
                    avsb = npool.tile([E, 512], F32, tag="avsb", bufs=2,
                                      name=f"avsb_{j}_{c}_{r}")
                    nc.vector.tensor_copy(out=avsb[:], in_=avs[r][0:E, :])
                    sums = npool.tile([1, 512], F32, tag="sums", bufs=2,
                                      name=f"sums_{j}_{c}_{r}")
                    nc.vector.tensor_copy(out=sums[:], in_=avs[r][E:W, :])
                    recip = npool.tile([1, 512], F32, tag="recip", bufs=1,
                                       name=f"recip_{j}_{c}_{r}")
                    nc.vector.reciprocal_approx_fast(recip[:], sums[:])
                    bc = npool.tile([E, 512], F32, tag="bc", bufs=2,
                                    name=f"bc_{j}_{c}_{r}")
                    nc.gpsimd.partition_broadcast(bc[:], recip[:])
                    nc.vector.tensor_mul(
                        outT[r * E:(r + 1) * E, j,
                             c * 512:(c + 1) * 512],
                        avsb[0:E, :], bc[:])

            # ------------- Phase A: K/Q proj + early scores + V ----------
            ex_c0 = {}
            with tc.tile_pool(name="stA", bufs=1) as stA, \
                 tc.tile_pool(name="psA", bufs=2, space="PSUM") as psA:

                def w_tile(nm, dram):
                    t = stA.tile([P, FT, DM], BF16, tag="w", bufs=2,
                                 name=nm)
                    src_ap = dram.ap().rearrange("(f p) n -> p f n", p=P)
                    for f in range(FT):
                        nc.sync.dma_start(out=t[:, f], in_=src_ap[:, f])
                    return t

                def x_tile(nm, dram, lo, n):
                    t = stA.tile([P, FT, n], BF16, tag="x", bufs=2,
                                 name=nm)
                    src_ap = (dram.ap()[:, lo:lo + n]
                              .rearrange("(f p) n -> p f n", p=P))
                    for f in range(FT):
                        nc.sync.dma_start(out=t[:, f], in_=src_ap[:, f])
                    return t

                def proj_qk(x_sb, w_sb, dst, bias, pos0, npos, tagn):
                    for m in range(FT):
                        pss = [psA.tile([P, 512], F32, tag="psa",
                                        name=f"psa_{tagn}_{m}_{c}")
                               for c in range(npos // 512)]
                        for f in range(FT):
                            for c in range(npos // 512):
                                nc.tensor.matmul(
                                    pss[c][:],
                                    lhsT=w_sb[:, f, m * P:(m + 1) * P],
                                    rhs=x_sb[:, f,
                                             c * 512:(c + 1) * 512],
                                    start=(f == 0), stop=(f == FT - 1))
                        for c in range(npos // 512):
                            nc.vector.tensor_scalar_add(
                                dst[:, m,
                                    pos0 + c * 512:pos0 + (c + 1) * 512],
                                pss[c][:], bias[:, m:m + 1])

                wk_sb = w_tile("wk", wk_d)
                for kc in range(2):
                    xk = x_tile(f"xk{kc}", xkT_d, kc * 512, 512)
                    proj_qk(xk, wk_sb, kT, bk_sb, kc * 512, 512, f"k{kc}")

                wq_sb = w_tile("wq", wq_d)
                xq0 = x_tile("xq0", xqT_d, 0, 512)
                proj_qk(xq0, wq_sb, qT, bq_sb, 0, 512, "q0")

                # first exp work: chunk 0, S-half 0 for head-pairs 0 and 1
                ex_j0 = [score_block(0, 0, 0, sb) for sb in range(4)]
                ex_c0[1] = [score_block(1, 0, 0, sb) for sb in range(4)]

                for kc in range(2, 4):
                    xk = x_tile(f"xk{kc}", xkT_d, kc * 512, 512)
                    proj_qk(xk, wk_sb, kT, bk_sb, kc * 512, 512, f"k{kc}")
                ex_j0 += [score_block(0, 0, 1, sb) for sb in range(4)]
                ex_c0[1] += [score_block(1, 0, 1, sb) for sb in range(4)]

                xq1 = x_tile("xq1", xqT_d, 512, 512)
                proj_qk(xq1, wq_sb, qT, bq_sb, 512, 512, "q1")

                wv_sb = w_tile("wv", wv_d)

                # V projection with interleaved: scores j2..j3 (chunk 0)
                # feeding the ACT exp stream, and the j0 AV accumulation
                # chasing v-tile completion.
                avs_j0 = [ps_av.tile([W, 512], F32, tag="av",
                                     name=f"av_0_0_{r}")
                          for r in range(2)]
                for vc in range(4):
                    xv_h = x_tile(f"xv{vc}", xvT_d, vc * 512, 512)
                    for ti in range(4):
                        t = vc * 4 + ti
                        pss = [psA.tile([P, 512], F32, tag="psa",
                                        name=f"psa_v{t}_{c}")
                               for c in range(2)]
                        for f in range(FT):
                            for c in range(2):
                                nc.tensor.matmul(
                                    pss[c][:],
                                    lhsT=xv_h[:, f, ti * P:(ti + 1) * P],
                                    rhs=wv_sb[:, f,
                                              c * 512:(c + 1) * 512],
                                    start=(f == 0), stop=False)
                        for c in range(2):
                            nc.tensor.matmul(
                                pss[c][:],
                                lhsT=ones_bf[:, 0:P],
                                rhs=bv_sb[:, c * 512:(c + 1) * 512],
                                start=False, stop=True)
                            nc.vector.tensor_copy(
                                out=v4[:, t, c * 8:(c + 1) * 8, 0:E],
                                in_=pss[c][:].rearrange(
                                    "p (h e) -> p h e", e=E))
                        # j0 AV chase: consume ex for s=t right away
                        ex = ex_j0[t // 2]
                        for r in range(2):
                            nc.tensor.matmul(
                                avs_j0[r][:],
                                lhsT=v4[:, t, r, :],
                                rhs=ex[:, r, t % 2, :],
                                start=(t == 0), stop=(t == NST - 1))
                    # emit j2..j3 score blocks between V quarters
                    if vc == 0:
                        ex_c0[2] = [score_block(2, 0, hf, sb)
                                    for hf in range(2) for sb in range(4)]
                    elif vc == 1:
                        ex_c0[3] = [score_block(3, 0, hf, sb)
                                    for hf in range(2) for sb in range(4)]

            # ------------- Phase B ------------------------------------
            with tc.tile_pool(name="bo_pool", bufs=1) as bop, \
                 tc.tile_pool(name="osb", bufs=2) as osb, \
                 tc.tile_pool(name="psC", bufs=1, space="PSUM") as psC:
                outT = bop.tile([P, FT, LL], BF16)
                wo_sb = bop.tile([P, FT, DM], BF16)
                nc.sync.dma_start(
                    out=wo_sb[:],
                    in_=wo_d.ap().rearrange("(f p) n -> p f n", p=P))

                normalize(0, 0, avs_j0, outT)

                def out_proj(t, c2):
                    ps = psC.tile([P, 512], F32, tag="psc",
                                  name=f"psc_{t}_{c2}")
                    for f in range(FT):
                        nc.tensor.matmul(
                            ps[:],
                            lhsT=outT[:, f, t * P:(t + 1) * P],
                            rhs=wo_sb[:, f, c2 * 512:(c2 + 1) * 512],
                            start=(f == 0), stop=False)
                    nc.tensor.matmul(
                        ps[:], lhsT=ones_bf[:, 0:P],
                        rhs=bo_sb[:, c2 * 512:(c2 + 1) * 512],
                        start=False, stop=True)
                    o_sb = osb.tile([P, 512], F32, tag="osb",
                                    name=f"osb_{t}_{c2}")
                    nc.vector.tensor_copy(out=o_sb[:], in_=ps[:])
                    nc.sync.dma_start(
                        out=out_d[t * P:(t + 1) * P,
                                  c2 * 512:(c2 + 1) * 512],
                        in_=o_sb[:])

                for c in range(2):
                    for j in range(NH2):
                        if c == 0 and j == 0:
                            continue  # done in phase A
                        if c == 0 and j in ex_c0:
                            ex_tiles = ex_c0.pop(j)
                        else:
                            ex_tiles = [score_block(j, c, hf, sb)
                                        for hf in range(2)
                                        for sb in range(4)]
                        avs = av_unit(j, c, ex_tiles)
                        normalize(j, c, avs, outT)
                        # interleave chunk-0 out-projection into the
                        # chunk-1 head loop (one L-tile per head pair)
                        if c == 1:
                            out_proj(j // 2, j % 2)

                for t in range(LL // P // 2, LL // P):
                    for c2 in range(2):
                        out_proj(t, c2)

    nc.finalize()
    _graph_cache["nc"] = nc
    return nc


def _install_profile_shim():
    """Provide antenv.axon_hooks (NTFF capture via libaxon_pjrt ctypes) when
    the image's antenv lacks it, and skip the artifact upload step."""
    import contextlib
    import ctypes
    import sys
    import types

    try:
        from antenv.axon_hooks import get_axon_ntff_profile_hook
        if get_axon_ntff_profile_hook() is not None:
            return
    except ImportError:
        pass

    so_path = "/opt/axon/libaxon_pjrt.so"
    try:
        lib = ctypes.CDLL(so_path)
    except OSError:
        return
    if not hasattr(lib, "axon_start_nrt_profile"):
        return
    lib.axon_start_nrt_profile.argtypes = [ctypes.POINTER(ctypes.c_int64),
                                           ctypes.c_size_t]
    lib.axon_start_nrt_profile.restype = ctypes.c_int64
    lib.axon_stop_nrt_profile.argtypes = [ctypes.c_char_p]
    lib.axon_stop_nrt_profile.restype = ctypes.c_int64

    @contextlib.contextmanager
    def _hook(output_dir, device_ids):
        import jax
        jax.devices()
        if device_ids:
            ids = (ctypes.c_int64 * len(device_ids))(*device_ids)
            rc = lib.axon_start_nrt_profile(ids, len(device_ids))
        else:
            rc = lib.axon_start_nrt_profile(None, 0)
        if rc != 0:
            raise RuntimeError(f"axon_start_nrt_profile rc={rc}")
        try:
            yield
        finally:
            n = lib.axon_stop_nrt_profile(str(output_dir).encode())
            print(f"profile: {n} file(s) written to {output_dir}",
                  file=sys.stderr)

    mod = types.ModuleType("antenv.axon_hooks")
    mod.get_axon_ntff_profile_hook = lambda: _hook
    mod.set_axon_ntff_profile_hook = lambda h: None
    sys.modules["antenv.axon_hooks"] = mod

    import concourse.bass_utils as bu
    bu.upload_artifacts = lambda tmpdir: str(tmpdir)


def kernel(queries, keys, values, Wq, bq, Wk, bk, Wv, bv, Wo, bo):
    from concourse.bass_utils import run_bass_kernel_spmd

    nc = _build_graph()
    bf = ml_dtypes.bfloat16

    wq_b = np.ascontiguousarray(np.asarray(Wq, np.float32).astype(bf))
    wk_b = np.ascontiguousarray(np.asarray(Wk, np.float32).astype(bf))
    wv_b = np.ascontiguousarray(np.asarray(Wv, np.float32).astype(bf))
    wo_b = np.ascontiguousarray(np.asarray(Wo, np.float32).astype(bf))
    bq_f = np.ascontiguousarray(np.asarray(bq, np.float32))
    bk_f = np.ascontiguousarray(np.asarray(bk, np.float32))
    bv_b = np.ascontiguousarray(np.asarray(bv, np.float32).astype(bf))
    bo_b = np.ascontiguousarray(np.asarray(bo, np.float32).astype(bf))

    qT = np.ascontiguousarray(
        np.transpose(np.asarray(queries, np.float32), (0, 2, 1)).astype(bf))
    kTt = np.ascontiguousarray(
        np.transpose(np.asarray(keys, np.float32), (0, 2, 1)).astype(bf))
    vTt = np.ascontiguousarray(
        np.transpose(np.asarray(values, np.float32), (0, 2, 1)).astype(bf))

    in_maps = []
    for core in range(NCORES):
        b, h = divmod(core, 2)
        in_maps.append({
            "xqT": np.ascontiguousarray(qT[b][:, h * LL:(h + 1) * LL]),
            "xkT": kTt[b],
            "xvT": vTt[b],
            "wq": wq_b, "wk": wk_b, "wv": wv_b, "wo": wo_b,
            "bq": bq_f, "bk": bk_f, "bv": bv_b, "bo": bo_b,
        })

    trace = bool(int(os.environ.get("KERNEL_PROFILE", "0")))
    if trace:
        _install_profile_shim()
    res = run_bass_kernel_spmd(nc, in_maps, core_ids=list(range(NCORES)),
                               trace=trace)
    kernel.last_results = res

    out = np.empty((B, L, DM), np.float32)
    for core in range(NCORES):
        b, h = divmod(core, 2)
        out[b, h * LL:(h + 1) * LL, :] = res.results[core]["out"]
    return out


kernel.last_results = None


# revision 27
# speedup vs baseline: 1.1932x; 1.0235x over previous
"""Trainium2 Bass kernel: dense multi-head attention layer (B=4, L=S=2048,
d_model=1024, 16 heads x 64).

Sharding: 8 cores = (batch b in 0..3) x (query-half in 0..1). Each core runs
the full 16-head attention for its 1024 query rows (K/V projection duplicated
between the two cores sharing a batch) plus the out-projection for those rows,
so no collectives are needed and each core emits complete output rows.

Layout trick: inputs are pre-transposed on the host to [d_model, pos] so every
matmul in the chain is natural for the PE array (contraction on partitions):
  kT = Wk.T @ XkT          (lhsT=Wk,  rhs=XkT)        -> [fout, pos]
  qT = Wq.T @ XqT          (lhsT=Wq,  rhs=XqT)        -> [fout, pos]
  v  = XvT.T @ Wv          (lhsT=XvT, rhs=Wv)         -> [pos, fout]
  scoresT_h = kT_h.T @ qT_h  (K=64, head pairs row-packed) -> [S, L]
  exp on ACT (no max subtraction needed: logits ~ N(0,1))
  avT_h = [v_h | 1].T @ expT_h  (K=S tiles, M=65)     -> [E+1, L], row 64 = sums
  outT_h = avT_h * (1/sums broadcast)                 -> [E, L]
  final = outT.T @ Wo (lhsT=outT, rhs=Wo)             -> [L, fout]  natural!
Biases enter via per-partition DVE adds (bq, bk) or K=1 ones-row matmuls
(bv, bo). Softmax scale 1/8 is folded into the ACT exp scale.

Schedule: the kernel's critical path is the ACT engine's exp stream (~317us
of activations) plus the PE projection ramp in front of it. Phase A is
ordered K(h0) -> Q(c0) -> scores(j0,c0,h0) -> K(h1) -> scores(j0,c0,h1) ->
Q(c1) -> V-proj, with scores j1..j2 (chunk 0) and the j0 AV accumulation
interleaved into the V projection (AV chases V-tile completion), so the ACT
exp pipeline starts at ~50us instead of ~200us and stays fed.

Notes from hardware iteration (do not re-try blindly):
- Two interleaved PSUM accumulation groups in one bank zero-region (e.g. an
  M=64 AV matmul plus an M=1 ones "sums" rider at col offset 64) are
  rejected/corrupted: start_tensor_calc clears the whole zero region.
- gpsimd.partition_broadcast of a DMA'd bias row + DVE tensor_add PSUM
  evacuation produced NaN on hardware (CoreSim-clean); the ones-row bias
  matmul form below is hardware-proven.
- Head-split sharding (8 heads/core, pairwise ReduceScatter of the
  row-parallel out-projection) shrinks the compute span to ~408us but the
  HBM collective + copy-out tail costs ~135us -> net loss vs this variant.
"""

import os

import ml_dtypes
import numpy as np

B, L, S, DM, H, E = 4, 2048, 2048, 1024, 16, 64
P = 128
FT = DM // P          # 8 feature tiles of 128
LL = L // 2           # 1024 query rows per core
NST = S // P          # 16 key/value position tiles
NH2 = H // 2          # 8 head pairs
W = E + 1             # v columns + ones column for the softmax denominator
NCORES = 8

_graph_cache = {}


def _build_graph():
    if "nc" in _graph_cache:
        return _graph_cache["nc"]

    import concourse.tile as tile
    from concourse import bacc, mybir

    BF16 = mybir.dt.bfloat16
    F32 = mybir.dt.float32
    Exp = mybir.ActivationFunctionType.Exp

    nc = bacc.Bacc("TRN2", target_bir_lowering=False, debug=False,
                   num_devices=NCORES)

    xqT_d = nc.declare_dram_parameter("xqT", [DM, LL], BF16, isOutput=False)
    xkT_d = nc.declare_dram_parameter("xkT", [DM, S], BF16, isOutput=False)
    xvT_d = nc.declare_dram_parameter("xvT", [DM, S], BF16, isOutput=False)
    wq_d = nc.declare_dram_parameter("wq", [DM, DM], BF16, isOutput=False)
    wk_d = nc.declare_dram_parameter("wk", [DM, DM], BF16, isOutput=False)
    wv_d = nc.declare_dram_parameter("wv", [DM, DM], BF16, isOutput=False)
    wo_d = nc.declare_dram_parameter("wo", [DM, DM], BF16, isOutput=False)
    bq_d = nc.declare_dram_parameter("bq", [DM], F32, isOutput=False)
    bk_d = nc.declare_dram_parameter("bk", [DM], F32, isOutput=False)
    bv_d = nc.declare_dram_parameter("bv", [DM], BF16, isOutput=False)
    bo_d = nc.declare_dram_parameter("bo", [DM], BF16, isOutput=False)
    out_d = nc.declare_dram_parameter("out", [LL, DM], F32, isOutput=True)

    HS = S // 2           # 1024 positions per streamed input half

    with tile.TileContext(nc) as tc:
        with tc.tile_pool(name="persist", bufs=1) as pp, \
             tc.tile_pool(name="norm", bufs=2) as npool, \
             tc.tile_pool(name="expp", bufs=16) as expp, \
             tc.tile_pool(name="ps_sc", bufs=2, space="PSUM") as ps_sc, \
             tc.tile_pool(name="ps_av", bufs=2, space="PSUM") as ps_av:
            qT = pp.tile([P, FT, LL], BF16)
            kT = pp.tile([P, FT, S], BF16)
            v_sb = pp.tile([P, NST, H * W], BF16)
            bq_sb = pp.tile([P, FT], F32)
            bk_sb = pp.tile([P, FT], F32)
            bv_sb = pp.tile([1, DM], BF16)
            bo_sb = pp.tile([1, DM], BF16)
            ones_bf = pp.tile([1, P], BF16)

            nc.vector.memset(ones_bf, 1.0)
            v4 = v_sb[:].rearrange("p s (h w) -> p s h w", w=W)
            nc.vector.memset(v4[:, :, :, E:W], 1.0)

            nc.sync.dma_start(out=bq_sb[:],
                              in_=bq_d.ap().rearrange("(f p) -> p f", p=P))
            nc.sync.dma_start(out=bk_sb[:],
                              in_=bk_d.ap().rearrange("(f p) -> p f", p=P))
            nc.sync.dma_start(out=bv_sb[:],
                              in_=bv_d.ap().rearrange("(a d) -> a d", a=1))
            nc.sync.dma_start(out=bo_sb[:],
                              in_=bo_d.ap().rearrange("(a d) -> a d", a=1))

            # PE warmup: dummy K=1 matmuls with no DMA dependency keep
            # the PE busy through the input-DMA ramp so the HAM clock
            # gate is already at 8/8 when the first real matmul issues.
            warm = ps_sc.tile([P, 2, 512], F32, tag="sc", name="warm")
            for wi in range(28):
                nc.tensor.matmul(warm[:, 0, 0:P],
                                 lhsT=ones_bf[:, 0:P],
                                 rhs=ones_bf[:, 0:P],
                                 start=True, stop=True)

            # Scores + exp for one (head-pair, chunk, half-of-S) block set.
            def score_block(j, c, half, sb):
                # Both heads' scores for one S-tile share ONE psum tile;
                # the two K=64 matmuls sit in disjoint row groups (base
                # partitions 0/64) so the PE runs them concurrently.
                ex = expp.tile([P, 2, 2, 512], BF16, tag="ex",
                               name=f"ex_{j}_{c}_{half}_{sb}")
                for si in range(2):
                    s = half * 8 + sb * 2 + si
                    scp = ps_sc.tile([P, 2, 512], F32, tag="sc",
                                     name=f"sc_{j}_{c}_{half}_{sb}_{si}")
                    for r in range(2):
                        nc.tensor.matmul(
                            scp[:, r, :],
                            lhsT=kT[r * E:(r + 1) * E, j,
                                    s * P:(s + 1) * P],
                            rhs=qT[r * E:(r + 1) * E, j,
                                   c * 512:(c + 1) * 512],
                            start=True, stop=True)
                    nc.scalar.activation(
                        out=ex[:, :, si, :],
                        in_=scp[:], func=Exp, scale=0.125)
                return ex

            # AV accumulation for one (head-pair, chunk) unit: M=65
            # matmuls ([v_h | 1] stationary) so row 64 of the PSUM bank
            # accumulates the softmax denominator for free.
            def av_unit(j, c, ex_tiles):
                avs = [ps_av.tile([W, 512], F32, tag="av",
                                  name=f"av_{j}_{c}_{r}")
                       for r in range(2)]
                for s in range(NST):
                    ex = ex_tiles[s // 2]
                    for r in range(2):
                        nc.tensor.matmul(
                            avs[r][:],
                            lhsT=v4[:, s, 2 * j + r, :],
                            rhs=ex[:, r, s % 2, :],
                            start=(s == 0), stop=(s == NST - 1))
                return avs

            def normalize(j, c, avs, outT):
                # NOTE: reciprocal_approx_fast (custom DVE op) gives NaN on
                # hardware when its input AP has a non-zero base partition
                # (sim-clean) — the sums row MUST be staged to a
                # base-partition-0 tile first.
                for r in range(2):
                    avsb = npool.tile([E, 512], F32, tag="avsb", bufs=2,
                                      name=f"avsb_{j}_{c}_{r}")
                    nc.vector.tensor_copy(out=avsb[:], in_=avs[r][0:E, :])
                    sums = npool.tile([1, 512], F32, tag="sums", bufs=1,
                                      name=f"sums_{j}_{c}_{r}")
                    nc.vector.tensor_copy(out=sums[:], in_=avs[r][E:W, :])
                    recip = npool.tile([1, 512], F32, tag="recip", bufs=1,
                                       name=f"recip_{j}_{c}_{r}")
                    nc.vector.reciprocal_approx_fast(recip[:], sums[:])
                    bc = npool.tile([E, 512], F32, tag="bc", bufs=1,
                                    name=f"bc_{j}_{c}_{r}")
                    nc.gpsimd.partition_broadcast(bc[:], recip[:])
                    nc.vector.tensor_mul(
                        outT[r * E:(r + 1) * E, j,
                             c * 512:(c + 1) * 512],
                        avsb[0:E, :], bc[:])

            # ------------- Phase A: K/Q proj + early scores + V ----------
            ex_c0 = {}
            with tc.tile_pool(name="stA", bufs=1) as stA, \
                 tc.tile_pool(name="psA", bufs=2, space="PSUM") as psA:

                def w_tile(nm, dram):
                    t = stA.tile([P, FT, DM], BF16, tag="w", bufs=2,
                                 name=nm)
                    src_ap = dram.ap().rearrange("(f p) n -> p f n", p=P)
                    for f in range(FT):
                        nc.sync.dma_start(out=t[:, f], in_=src_ap[:, f])
                    return t

                def x_tile(nm, dram, lo, n):
                    t = stA.tile([P, FT, n], BF16, tag="x", bufs=2,
                                 name=nm)
                    src_ap = (dram.ap()[:, lo:lo + n]
                              .rearrange("(f p) n -> p f n", p=P))
                    for f in range(FT):
                        nc.sync.dma_start(out=t[:, f], in_=src_ap[:, f])
                    return t

                def proj_qk(x_sb, w_sb, dst, bias, pos0, npos, tagn):
                    for m in range(FT):
                        pss = [psA.tile([P, 512], F32, tag="psa",
                                        name=f"psa_{tagn}_{m}_{c}")
                               for c in range(npos // 512)]
                        for f in range(FT):
                            for c in range(npos // 512):
                                nc.tensor.matmul(
                                    pss[c][:],
                                    lhsT=w_sb[:, f, m * P:(m + 1) * P],
                                    rhs=x_sb[:, f,
                                             c * 512:(c + 1) * 512],
                                    start=(f == 0), stop=(f == FT - 1))
                        for c in range(npos // 512):
                            nc.vector.tensor_scalar_add(
                                dst[:, m,
                                    pos0 + c * 512:pos0 + (c + 1) * 512],
                                pss[c][:], bias[:, m:m + 1])

                wk_sb = w_tile("wk", wk_d)
                for kc in range(2):
                    xk = x_tile(f"xk{kc}", xkT_d, kc * 512, 512)
                    proj_qk(xk, wk_sb, kT, bk_sb, kc * 512, 512, f"k{kc}")

                wq_sb = w_tile("wq", wq_d)
                xq0 = x_tile("xq0", xqT_d, 0, 512)
                proj_qk(xq0, wq_sb, qT, bq_sb, 0, 512, "q0")

                # first exp work: chunk 0, S-half 0 for head-pairs 0 and 1
                ex_j0 = [score_block(0, 0, 0, sb) for sb in range(4)]
                ex_c0[1] = [score_block(1, 0, 0, sb) for sb in range(4)]

                for kc in range(2, 4):
                    xk = x_tile(f"xk{kc}", xkT_d, kc * 512, 512)
                    proj_qk(xk, wk_sb, kT, bk_sb, kc * 512, 512, f"k{kc}")
                ex_j0 += [score_block(0, 0, 1, sb) for sb in range(4)]
                ex_c0[1] += [score_block(1, 0, 1, sb) for sb in range(4)]

                xq1 = x_tile("xq1", xqT_d, 512, 512)
                proj_qk(xq1, wq_sb, qT, bq_sb, 512, 512, "q1")

                wv_sb = w_tile("wv", wv_d)

                # V projection with interleaved: scores j2..j3 (chunk 0)
                # feeding the ACT exp stream, and the j0 AV accumulation
                # chasing v-tile completion.
                avs_j0 = [ps_av.tile([W, 512], F32, tag="av",
                                     name=f"av_0_0_{r}")
                          for r in range(2)]
                for vc in range(4):
                    xv_h = x_tile(f"xv{vc}", xvT_d, vc * 512, 512)
                    for ti in range(4):
                        t = vc * 4 + ti
                        pss = [psA.tile([P, 512], F32, tag="psa",
                                        name=f"psa_v{t}_{c}")
                               for c in range(2)]
                        for f in range(FT):
                            for c in range(2):
                                nc.tensor.matmul(
                                    pss[c][:],
                                    lhsT=xv_h[:, f, ti * P:(ti + 1) * P],
                                    rhs=wv_sb[:, f,
                                              c * 512:(c + 1) * 512],
                                    start=(f == 0), stop=False)
                        for c in range(2):
                            nc.tensor.matmul(
                                pss[c][:],
                                lhsT=ones_bf[:, 0:P],
                                rhs=bv_sb[:, c * 512:(c + 1) * 512],
                                start=False, stop=True)
                            nc.vector.tensor_copy(
                                out=v4[:, t, c * 8:(c + 1) * 8, 0:E],
                                in_=pss[c][:].rearrange(
                                    "p (h e) -> p h e", e=E))
                        # j0 AV chase: consume ex for s=t right away
                        ex = ex_j0[t // 2]
                        for r in range(2):
                            nc.tensor.matmul(
                                avs_j0[r][:],
                                lhsT=v4[:, t, r, :],
                                rhs=ex[:, r, t % 2, :],
                                start=(t == 0), stop=(t == NST - 1))
                    # emit j2..j3 score blocks between V quarters
                    if vc == 0:
                        ex_c0[2] = [score_block(2, 0, hf, sb)
                                    for hf in range(2) for sb in range(4)]
                    elif vc == 1:
                        ex_c0[3] = [score_block(3, 0, hf, sb)
                                    for hf in range(2) for sb in range(4)]

            # ------------- Phase B ------------------------------------
            with tc.tile_pool(name="bo_pool", bufs=1) as bop, \
                 tc.tile_pool(name="osb", bufs=2) as osb, \
                 tc.tile_pool(name="psC", bufs=2, space="PSUM") as psC:
                outT = bop.tile([P, FT, LL], BF16)
                wo_sb = bop.tile([P, FT, DM], BF16)
                nc.sync.dma_start(
                    out=wo_sb[:],
                    in_=wo_d.ap().rearrange("(f p) n -> p f n", p=P))

                normalize(0, 0, avs_j0, outT)

                def out_proj(t, c2):
                    ps = psC.tile([P, 512], F32, tag="psc",
                                  name=f"psc_{t}_{c2}")
                    for f in range(FT):
                        nc.tensor.matmul(
                            ps[:],
                            lhsT=outT[:, f, t * P:(t + 1) * P],
                            rhs=wo_sb[:, f, c2 * 512:(c2 + 1) * 512],
                            start=(f == 0), stop=False)
                    nc.tensor.matmul(
                        ps[:], lhsT=ones_bf[:, 0:P],
                        rhs=bo_sb[:, c2 * 512:(c2 + 1) * 512],
                        start=False, stop=True)
                    o_sb = osb.tile([P, 512], F32, tag="osb",
                                    name=f"osb_{t}_{c2}")
                    nc.vector.tensor_copy(out=o_sb[:], in_=ps[:])
                    nc.sync.dma_start(
                        out=out_d[t * P:(t + 1) * P,
                                  c2 * 512:(c2 + 1) * 512],
                        in_=o_sb[:])

                for c in range(2):
                    for j in range(NH2):
                        if c == 0 and j == 0:
                            continue  # done in phase A
                        if c == 0 and j in ex_c0:
                            ex_tiles = ex_c0.pop(j)
                        else:
                            ex_tiles = [score_block(j, c, hf, sb)
                                        for hf in range(2)
                                        for sb in range(4)]
                        avs = av_unit(j, c, ex_tiles)
                        normalize(j, c, avs, outT)
                        # interleave chunk-0 out-projection into the
                        # chunk-1 head loop (one L-tile per head pair)
                        if c == 1:
                            out_proj(j // 2, j % 2)

                for t in range(LL // P // 2, LL // P):
                    for c2 in range(2):
                        out_proj(t, c2)

    nc.finalize()
    _graph_cache["nc"] = nc
    return nc


def _install_profile_shim():
    """Provide antenv.axon_hooks (NTFF capture via libaxon_pjrt ctypes) when
    the image's antenv lacks it, and skip the artifact upload step."""
    import contextlib
    import ctypes
    import sys
    import types

    try:
        from antenv.axon_hooks import get_axon_ntff_profile_hook
        if get_axon_ntff_profile_hook() is not None:
            return
    except ImportError:
        pass

    so_path = "/opt/axon/libaxon_pjrt.so"
    try:
        lib = ctypes.CDLL(so_path)
    except OSError:
        return
    if not hasattr(lib, "axon_start_nrt_profile"):
        return
    lib.axon_start_nrt_profile.argtypes = [ctypes.POINTER(ctypes.c_int64),
                                           ctypes.c_size_t]
    lib.axon_start_nrt_profile.restype = ctypes.c_int64
    lib.axon_stop_nrt_profile.argtypes = [ctypes.c_char_p]
    lib.axon_stop_nrt_profile.restype = ctypes.c_int64

    @contextlib.contextmanager
    def _hook(output_dir, device_ids):
        import jax
        jax.devices()
        if device_ids:
            ids = (ctypes.c_int64 * len(device_ids))(*device_ids)
            rc = lib.axon_start_nrt_profile(ids, len(device_ids))
        else:
            rc = lib.axon_start_nrt_profile(None, 0)
        if rc != 0:
            raise RuntimeError(f"axon_start_nrt_profile rc={rc}")
        try:
            yield
        finally:
            n = lib.axon_stop_nrt_profile(str(output_dir).encode())
            print(f"profile: {n} file(s) written to {output_dir}",
                  file=sys.stderr)

    mod = types.ModuleType("antenv.axon_hooks")
    mod.get_axon_ntff_profile_hook = lambda: _hook
    mod.set_axon_ntff_profile_hook = lambda h: None
    sys.modules["antenv.axon_hooks"] = mod

    import concourse.bass_utils as bu
    bu.upload_artifacts = lambda tmpdir: str(tmpdir)


def kernel(queries, keys, values, Wq, bq, Wk, bk, Wv, bv, Wo, bo):
    from concourse.bass_utils import run_bass_kernel_spmd

    nc = _build_graph()
    bf = ml_dtypes.bfloat16

    wq_b = np.ascontiguousarray(np.asarray(Wq, np.float32).astype(bf))
    wk_b = np.ascontiguousarray(np.asarray(Wk, np.float32).astype(bf))
    wv_b = np.ascontiguousarray(np.asarray(Wv, np.float32).astype(bf))
    wo_b = np.ascontiguousarray(np.asarray(Wo, np.float32).astype(bf))
    bq_f = np.ascontiguousarray(np.asarray(bq, np.float32))
    bk_f = np.ascontiguousarray(np.asarray(bk, np.float32))
    bv_b = np.ascontiguousarray(np.asarray(bv, np.float32).astype(bf))
    bo_b = np.ascontiguousarray(np.asarray(bo, np.float32).astype(bf))

    qT = np.ascontiguousarray(
        np.transpose(np.asarray(queries, np.float32), (0, 2, 1)).astype(bf))
    kTt = np.ascontiguousarray(
        np.transpose(np.asarray(keys, np.float32), (0, 2, 1)).astype(bf))
    vTt = np.ascontiguousarray(
        np.transpose(np.asarray(values, np.float32), (0, 2, 1)).astype(bf))

    in_maps = []
    for core in range(NCORES):
        b, h = divmod(core, 2)
        in_maps.append({
            "xqT": np.ascontiguousarray(qT[b][:, h * LL:(h + 1) * LL]),
            "xkT": kTt[b],
            "xvT": vTt[b],
            "wq": wq_b, "wk": wk_b, "wv": wv_b, "wo": wo_b,
            "bq": bq_f, "bk": bk_f, "bv": bv_b, "bo": bo_b,
        })

    trace = bool(int(os.environ.get("KERNEL_PROFILE", "0")))
    if trace:
        _install_profile_shim()
    res = run_bass_kernel_spmd(nc, in_maps, core_ids=list(range(NCORES)),
                               trace=trace)
    kernel.last_results = res

    out = np.empty((B, L, DM), np.float32)
    for core in range(NCORES):
        b, h = divmod(core, 2)
        out[b, h * LL:(h + 1) * LL, :] = res.results[core]["out"]
    return out


kernel.last_results = None
